# revision 1
# baseline (speedup 1.0000x reference)
"""Trainium2 Bass kernel for nn_MHA_29008209117536.

MHA with a temporal-bias MLP:
  q = (Xq Wq) split-heads; k/v from Xk; scores = qk^T/8 + bias(T); softmax; out = (attn v) Wp

Key observations baked into the kernel:
  * The temporal-bias MLP collapses: t = 1/log(e+T) > 0 always, so
    leaky_relu(t * Wt1) = t * (Wt1 if Wt1>=0 else 0.2*Wt1) elementwise, and
    bias = C * t with scalar C = sum(Wt2 * leaky(Wt1)).  Computed on host.
  * Sharding: data-parallel over batch. B=8 == n_cores; core b handles batch b.
  * All matmuls run in a "T-space" layout that needs NO on-device transposes:
      - host passes XqT, XkT (the [D,S] transposes; free on host)
      - QT = Wq^T Xq^T and KT likewise:  lhsT=Wq-tile, rhs=XqT  (contract d_in)
      - V natural [S,D]:                 lhsT=XkT-tile, rhs=Wv
      - ST_h = K_h Q_h^T ([s_k, s_q]):   lhsT=KT_h, rhs=QT_h    (contract d_k)
      - the temporal bias is ADDED ON THE PE: a second accumulating matmul
        (C*I) @ (1/ln(e+T^T)) adds the full-rank bias tile into the scores
        PSUM bank (cheaper than an elementwise DVE add: 213ns vs ~2.6us/head)
      - softmax runs over s_k (partition dim): plain exp (no max-sub; scores
        are O(10) so fp32 exp is exact enough), denominators come free from a
        ones-column appended to V in the AV matmul.
      - OT_h = V_h^T P_h^T ([dk, s_q]):  lhsT=[V_h|1], rhs=exp(ST_h)
      - out  = OT^T Wp ([s_q, d]):       lhsT=OT_h, rhs=Wp_h    (contract dk)
  * float32r matmul dtype: 1 cycle/row at N=512 (4x faster than fp32),
    ~19-bit mantissa (measured end-to-end rel err ~3e-4).
"""

import numpy as np

import concourse.bass as bass
import concourse.mybir as mybir
import concourse.tile as tile
from concourse import bacc
from concourse.bass_utils import run_bass_kernel_spmd

F32 = mybir.dt.float32
F32R = mybir.dt.float32r
AF = mybir.ActivationFunctionType

B, S, D, H, TB = 8, 512, 512, 8, 64
DK = D // H          # 64
P = 128              # partitions
NT = S // P          # 4 tiles of 128 along any 512 dim
N_CORES = 8


def build_nc(C: float, use_bias: bool):
    nc = bacc.Bacc("TRN2", target_bir_lowering=False, debug=False,
                   num_devices=N_CORES)

    xqT = nc.dram_tensor("xqT", [D, S], F32, kind="ExternalInput").ap()
    xkT = nc.dram_tensor("xkT", [D, S], F32, kind="ExternalInput").ap()
    b16 = nc.dram_tensor("binv16", [S, S], mybir.dt.uint16,
                         kind="ExternalInput").ap()
    qab = nc.dram_tensor("qab", [P, 2], F32, kind="ExternalInput").ap()
    # wq/wk arrive host-blocked as [m, p, kt, d] so each m-column block is
    # one DMA with 2KB-contiguous per-partition reads
    wq = nc.dram_tensor("wq", [NT, P, NT, P], F32, kind="ExternalInput").ap()
    wk = nc.dram_tensor("wk", [NT, P, NT, P], F32, kind="ExternalInput").ap()
    wv = nc.dram_tensor("wv", [D, D], F32, kind="ExternalInput").ap()
    wp = nc.dram_tensor("wp", [D, D], F32, kind="ExternalInput").ap()
    ci = nc.dram_tensor("ci", [P, P], F32, kind="ExternalInput").ap()
    out = nc.dram_tensor("out", [S, D], F32, kind="ExternalOutput").ap()

    # [din, dout] -> [p, kt, dout]: partition = din within k-tile
    wq_t = wq.bitcast(F32R)
    wk_t = wk.bitcast(F32R)
    wv_t = wv.bitcast(F32R).rearrange("(kt p) d -> p kt d", p=P)
    wp_t = wp.bitcast(F32R).rearrange("(kt p) d -> p kt d", p=P)
    xqT_t = xqT.bitcast(F32R).rearrange("(kt p) s -> p kt s", p=P)
    xkT_t = xkT.bitcast(F32R).rearrange("(kt p) s -> p kt s", p=P)
    b16_t = b16.rearrange("(kt p) s -> p kt s", p=P)
    out_t = out.rearrange("(st p) d -> p st d", p=P)

    with tile.TileContext(nc) as tc:
        with (
            tc.tile_pool(name="const", bufs=1) as cpool,
            tc.tile_pool(name="work", bufs=2) as wpool,
            tc.tile_pool(name="otn", bufs=H) as opool,
            tc.tile_pool(name="dram", bufs=1, space="DRAM") as dpool,
            tc.tile_pool(name="pj", bufs=2, space="PSUM") as pjp,
            tc.tile_pool(name="st", bufs=2, space="PSUM") as stp,
            tc.tile_pool(name="av", bufs=2, space="PSUM") as avp,
        ):
            wq_sb = cpool.tile([P, NT, D], F32R, tag="wq")
            xq_sb = cpool.tile([P, NT, S], F32R, tag="xq")
            wk_sb = cpool.tile([P, NT, D], F32R, tag="wk")
            xk_sb = cpool.tile([P, NT, S], F32R, tag="xk")
            wv_sb = cpool.tile([P, NT, D], F32R, tag="wv")
            wp_sb4 = cpool.tile([P, NT, D], F32R, tag="wp")
            qt_sb = cpool.tile([P, NT, S], F32R, tag="qt")
            kt_sb = cpool.tile([P, NT, S], F32R, tag="kt")
            vb_sb = cpool.tile([P, NT, H * (DK + 1)], F32R, tag="vb")
            rb_sb = cpool.tile([DK, H, S], F32, tag="rb")
            rscr = dpool.tile([H, S], F32, tag="rscr")
            if use_bias:
                ci_sb = cpool.tile([P, P], F32R, tag="ci")
                b16_sb = cpool.tile([P, NT, S], mybir.dt.uint16, tag="b16")
                qab_sb = cpool.tile([P, 2], F32, tag="qab")
                binv_sb = cpool.tile([P, NT, S], F32R, tag="binv")

            # ---- input DMAs, ordered by when compute needs them ----
            # wq/wk stream in m-column blocks so the first QT/KT groups (and
            # with them the first heads) start after ~3MB instead of ~4.5MB
            if use_bias:
                nc.sync.dma_start(out=ci_sb, in_=ci.bitcast(F32R))
                nc.sync.dma_start(out=qab_sb, in_=qab)
            for kt in range(NT):
                nc.sync.dma_start(out=xq_sb[:, kt, :], in_=xqT_t[:, kt, :])
                nc.sync.dma_start(out=xk_sb[:, kt, :], in_=xkT_t[:, kt, :])
            if use_bias:
                for kt in range(NT):
                    nc.sync.dma_start(out=b16_sb[:, kt, :], in_=b16_t[:, kt, :])
            nc.sync.dma_start(out=wq_sb[:, :, 0:P], in_=wq_t[0])
            nc.sync.dma_start(out=wk_sb[:, :, 0:P], in_=wk_t[0])
            for kt in range(NT):
                nc.sync.dma_start(out=wv_sb[:, kt, :], in_=wv_t[:, kt, :])
            for m in range(1, NT):
                nc.sync.dma_start(out=wq_sb[:, :, m * P:(m + 1) * P],
                                  in_=wq_t[m])
                nc.sync.dma_start(out=wk_sb[:, :, m * P:(m + 1) * P],
                                  in_=wk_t[m])
            for kt in range(NT):
                nc.sync.dma_start(out=wp_sb4[:, kt, :], in_=wp_t[:, kt, :])

            # ---- temporal bias: binv = 1/ln(e + T^T), u16-dequantized ----
            # host passes binv16 = round((binv-lo)/(hi-lo)*65535) and
            # qab = [a, b] with a=(hi-lo)/65535, b=lo; C rides in CI
            if use_bias:
                with nc.allow_low_precision(
                        reason="f32r bias term; ~19-bit mantissa is plenty"):
                    nc.vector.tensor_scalar(
                        out=binv_sb, in0=b16_sb,
                        scalar1=qab_sb[:, 0:1], scalar2=qab_sb[:, 1:2],
                        op0=mybir.AluOpType.mult, op1=mybir.AluOpType.add)

            # ones columns of the [V | 1] blocks (memset can't emit f32r;
            # bounce through an f32 tile and let the DVE copy convert)
            vb_heads = vb_sb.rearrange("p kt (h c) -> p kt h c", c=DK + 1)
            one_sb = cpool.tile([P, NT, H, 1], F32, tag="ones")
            nc.vector.memset(one_sb, 1.0)
            nc.vector.tensor_copy(out=vb_heads[:, :, :, DK:DK + 1], in_=one_sb)

            # ---- projection group emitters (interleaved into head loop) ----
            def emit_qt(m):
                ps = pjp.tile([P, S], F32, tag="pj", name=f"qtps{m}")
                for kt in range(NT):
                    nc.tensor.matmul(ps,
                                     wq_sb[:, kt, m * P:(m + 1) * P],
                                     xq_sb[:, kt, :],
                                     start=(kt == 0), stop=(kt == NT - 1))
                nc.vector.tensor_copy(out=qt_sb[:, m, :], in_=ps)

            def emit_kt(m):
                ps = pjp.tile([P, S], F32, tag="pj", name=f"ktps{m}")
                for kt in range(NT):
                    nc.tensor.matmul(ps,
                                     wk_sb[:, kt, m * P:(m + 1) * P],
                                     xk_sb[:, kt, :],
                                     start=(kt == 0), stop=(kt == NT - 1))
                nc.vector.tensor_copy(out=kt_sb[:, m, :], in_=ps)

            def emit_v():
                for sv in range(NT):
                    ps = pjp.tile([P, S], F32, tag="pj", name=f"vps{sv}")
                    for kt in range(NT):
                        nc.tensor.matmul(ps,
                                         xk_sb[:, kt, sv * P:(sv + 1) * P],
                                         wv_sb[:, kt, :],
                                         start=(kt == 0), stop=(kt == NT - 1))
                    # scatter dk-columns into the [V | 1] per-head blocks
                    nc.vector.tensor_copy(
                        out=vb_heads[:, sv, :, 0:DK],
                        in_=ps.rearrange("p (h c) -> p h c", c=DK))

            # ---- per-head attention, software-pipelined emission ----
            # The PE drains its queue in order, so AV matmuls of head h-1 are
            # emitted AFTER the ST matmuls of head h: the PE never stalls on
            # an exp. Scores PSUM is two 2-bank tiles per head (st bufs=2).
            # Output projection works on head PAIRS: the odd head's
            # normalized OT is partition-shifted 0:64 -> 64:128 into a pair
            # tile by an SBUF->SBUF DMA, so OUT needs 16 K=128 matmuls.
            pairs = [cpool.tile([P, S], F32R, tag=f"pair{t}", name=f"pair{t}")
                     for t in range(NT)]
            out_ps = []

            def emit_st(h):
                hm, hp = h // 2, (h % 2) * DK
                pts = []
                for half in range(2):
                    stw = stp.tile([P, 2, S], F32, tag="st",
                                   name=f"st_{h}_{half}")
                    for jj in range(2):
                        j = half * 2 + jj
                        nc.tensor.matmul(
                            stw[:, jj, :],
                            kt_sb[hp:hp + DK, hm, j * P:(j + 1) * P],
                            qt_sb[hp:hp + DK, hm, :],
                            start=True, stop=not use_bias)
                        if use_bias:
                            # scores += (C*I) @ binv — full-rank bias on PE
                            nc.tensor.matmul(
                                stw[:, jj, :], ci_sb, binv_sb[:, j, :],
                                start=False, stop=True)
                    pt = wpool.tile([P, 2, S], F32R, tag="pt", bufs=4,
                                    name=f"pt_{h}_{half}")
                    nc.scalar.activation(out=pt, in_=stw, func=AF.Exp,
                                         scale=1.0)
                    pts.append(pt)
                return pts

            def emit_av(h, pts):
                av = avp.tile([DK + 1, S], F32, tag="av", name=f"av_{h}")
                for kt in range(NT):
                    nc.tensor.matmul(
                        av,
                        vb_sb[:, kt, h * (DK + 1):(h + 1) * (DK + 1)],
                        pts[kt // 2][:, kt % 2, :],
                        start=(kt == 0), stop=(kt == NT - 1))
                # stage [OT~ | sums] to SBUF (frees the av bank fast), then
                # reciprocal the sums row and broadcast it over dk partitions
                # via a DRAM bounce. The last head skips the staging copy —
                # nothing waits on its PSUM slot, and the copy would sit on
                # the critical tail.
                last = h == 6
                otu = wpool.tile([DK + 1, S], F32, tag="otu", bufs=3,
                                 name=f"otu_{h}")
                if not last:
                    nc.vector.tensor_copy(out=otu, in_=av)
                    nc.vector.reciprocal(out=otu[DK:DK + 1, :],
                                         in_=otu[DK:DK + 1, :])
                    src_ot = otu
                else:
                    nc.vector.reciprocal(out=otu[DK:DK + 1, :],
                                         in_=av[DK:DK + 1, :])
                    src_ot = av
                nc.sync.dma_start(out=rscr[h:h + 1, :], in_=otu[DK:DK + 1, :])
                nc.sync.dma_start(
                    out=rb_sb[:, h, :],
                    in_=rscr[h:h + 1, :].to_broadcast((DK, S)))
                t = h // 2
                if h % 2 == 0:
                    nc.vector.tensor_mul(out=pairs[t][0:DK, :],
                                         in0=src_ot[0:DK, :],
                                         in1=rb_sb[:, h, :])
                else:
                    otn = opool.tile([DK, S], F32R, tag="otn",
                                     name=f"otn_{h}")
                    nc.vector.tensor_mul(out=otn, in0=src_ot[0:DK, :],
                                         in1=rb_sb[:, h, :])
                    # partition-shift the odd head into rows 64:128
                    nc.sync.dma_start(out=pairs[t][DK:P, :], in_=otn)

            def emit_out_pair(t, st_list):
                # advance OUT s-groups by this pair's contribution; only two
                # groups fit in the pj PSUM slots during the head loop
                for st_ in st_list:
                    if t == 0:
                        while len(out_ps) <= st_:
                            out_ps.append(None)
                        out_ps[st_] = pjp.tile([P, S], F32, tag="pj",
                                               name=f"out_ps{st_}")
                    nc.tensor.matmul(out_ps[st_],
                                     pairs[t][:, st_ * P:(st_ + 1) * P],
                                     wp_sb4[:, t, :],
                                     start=(t == 0), stop=(t == NT - 1))

            # odd head first within each pair (the even head's direct write
            # finalizes the pair tile, keeping the partition shift off the
            # critical tail); QT/KT of pair p emitted just before its heads;
            # AV of head x emitted after the NEXT head's ST so the in-order
            # PE queue never stalls on an exp.
            emit_qt(0)
            emit_kt(0)
            pts1 = emit_st(1)
            pts0 = emit_st(0)
            emit_v()
            emit_av(1, pts1)
            pending = (0, pts0)          # even head awaiting AV
            for p in (1, 2, 3):
                emit_qt(p)
                emit_kt(p)
                if p == 3:
                    # the pj slots are free of projection groups only now;
                    # pairs 0/1 are long done, so these never stall the PE
                    emit_out_pair(0, (0, 1))
                    emit_out_pair(1, (0, 1))
                pts_odd = emit_st(2 * p + 1)
                emit_av(*pending)        # even head of pair p-1
                pts_even = emit_st(2 * p)
                emit_av(2 * p + 1, pts_odd)
                pending = (2 * p, pts_even)
            emit_av(*pending)            # h6, finalizes pair 3

            # s2/s3 accumulate in a 2-bank tile from the draining st pool so
            # they overlap the s0/s1 groups instead of waiting for their slots
            emit_out_pair(2, (0, 1))
            out23 = stp.tile([P, 2, S], F32, tag="st", name="out23")
            for t in range(NT):
                for i, st_ in enumerate((2, 3)):
                    nc.tensor.matmul(out23[:, i, :],
                                     pairs[t][:, st_ * P:(st_ + 1) * P],
                                     wp_sb4[:, t, :],
                                     start=(t == 0), stop=(t == NT - 1))
            emit_out_pair(3, (0, 1))

            for st_ in (0, 1):
                osb = wpool.tile([P, S], F32, tag="osb", bufs=4,
                                 name=f"osb{st_}")
                nc.vector.tensor_copy(out=osb, in_=out_ps[st_])
                nc.sync.dma_start(out=out_t[:, st_, :], in_=osb)
            for i, st_ in enumerate((2, 3)):
                osb = wpool.tile([P, S], F32, tag="osb", bufs=4,
                                 name=f"osb{st_}")
                nc.scalar.activation(out=osb, in_=out23[:, i, :],
                                     func=AF.Copy, bias=0.0)
                nc.sync.dma_start(out=out_t[:, st_, :], in_=osb)

    nc.compile()
    return nc


_CACHE = {}


def _get_nc(C: float, use_bias: bool):
    key = (round(C, 12), use_bias)
    if key not in _CACHE:
        _CACHE[key] = build_nc(C, use_bias)
    return _CACHE[key]


def prepare(inputs: dict):
    q = np.ascontiguousarray(np.asarray(inputs["query_input"], dtype=np.float32))
    k = np.ascontiguousarray(np.asarray(inputs["key_input"], dtype=np.float32))
    t = np.ascontiguousarray(np.asarray(inputs["batch_temporal_mat"], dtype=np.float32))
    Wq = np.asarray(inputs["Wq"], dtype=np.float32)
    Wk = np.asarray(inputs["Wk"], dtype=np.float32)
    Wv = np.asarray(inputs["Wv"], dtype=np.float32)
    Wp = np.asarray(inputs["Wp"], dtype=np.float32)
    Wt1 = np.asarray(inputs["Wt1"], dtype=np.float32)[0]
    Wt2 = np.asarray(inputs["Wt2"], dtype=np.float32)[:, 0]

    C = float(np.sum(Wt2 * np.where(Wt1 >= 0.0, Wt1, 0.2 * Wt1), dtype=np.float64))
    use_bias = abs(C) > 1e-20
    wq_s = np.ascontiguousarray(
        (Wq * np.float32(0.125)).reshape(NT, P, NT, P).transpose(2, 1, 0, 3))
    wk_c = np.ascontiguousarray(Wk.reshape(NT, P, NT, P).transpose(2, 1, 0, 3))
    wv_c = np.ascontiguousarray(Wv)
    wp_c = np.ascontiguousarray(Wp)
    ci = np.ascontiguousarray(np.eye(P, dtype=np.float32) * np.float32(C))

    nc = _get_nc(C, use_bias)

    binv = 1.0 / np.log(np.float32(np.e) + t)          # [B, S, S]
    lo = float(binv.min())
    hi = float(binv.max())
    scale = (hi - lo) / 65535.0 if hi > lo else 1.0
    b16 = np.round((binv - lo) / scale).astype(np.uint16)
    qab = np.broadcast_to(
        np.array([[scale, lo]], dtype=np.float32), (P, 2)).copy()

    in_maps = []
    for b in range(N_CORES):
        in_maps.append({
            "xqT": np.ascontiguousarray(q[b].T),
            "xkT": np.ascontiguousarray(k[b].T),
            "binv16": np.ascontiguousarray(b16[b].T),
            "qab": qab,
            "wq": wq_s,
            "wk": wk_c,
            "wv": wv_c,
            "wp": wp_c,
            "ci": ci,
        })
    return nc, in_maps


def kernel(**inputs) -> np.ndarray:
    nc, in_maps = prepare(inputs)
    res = run_bass_kernel_spmd(nc, in_maps, list(range(N_CORES)), trace=False)
    return np.stack([res.results[b]["out"] for b in range(N_CORES)], axis=0)

